# revision 22
# baseline (speedup 1.0000x reference)
"""Trainium2 Bass kernel for CausalNCMomentumAttention (linear attention,
causal + non-causal branches).

Shapes (hardcoded): N=2, L=8192, H=8, E=M=64, fp32 in/out.
Sharding: 8 cores; core i handles batch n = i//4, heads 2*(i%4)..+1.

The PE's HAM clock gate only counts full-row (128-contraction) matmuls
as busy: any 64-row matmul stream runs at the cold 1.2 GHz clock forever
(measured).  So EVERY matmul contracts over 128 partitions, with the
two heads packed by block-diagonal weights and col-tiled outputs
((128,64)/(128,128) keep HAM warm; (64,x) does not).  Dummy "drip"
matmuls and two warm-up bursts hold the clock gate open through sparse
stream sections.

Work is tiled in 256-row SUPERCHUNKS (two 128-row PE chunks c0,c1):

  qt2 [128,L]    stacked heads: rows 0:64 = Qf_h0^T, 64:128 = Qf_h1^T
  ktp [128,2,L]  ktp[:,h] has Kf_h^T in rows h*64:.., ZEROS elsewhere
  ks/v2m [128,NCH,128]  chunk-major [h0 | h1] blocks of Kf and V*mask

  D:     4 col-tiled MMs accumulate the 256-row state update into the
         diagonal blocks of one persistent PSUM tile (off-blocks zeroed
         once); ONE fp32 SP add + ONE bf16 block-diag snapshot SS[sc]
         per superchunk.
  at:    per head: ktp[c0]^T @ qt2[c0:c2] (N=256: the cross block
         [s in c0, l in c1] IS c1's inter term w.r.t. c0) and
         ktp[c1]^T @ qt2[c1] (N=128); ONE DVE evac per superchunk with
         the composite mask [tri | ones | tri] -> bf16 at_sb.
  inter: ONE MM per superchunk: SS[sc] (block-diag) @ qt2 (N=256).
  intra: per head: v2m[c0,h]^T @ at_sb[h,0:256] (N=256, covers diag c0
         AND the cross term) + v2m[c1,h]^T @ at_sb[h,256:384] into the
         l-c1 half; col-tiled into rows h*64:(h+1)*64.
  nc:    SfinBD (block-diag final state) @ qt2, N=512 blocks.

Outputs are unnormalized bf16 numerators ([128,NCH,128] causal,
[128,L] non-causal, rows = stacked (h,m)); the host applies the feature
map (elu+1, fp32), computes both denominators (fp32 cumsum/einsum) and
the final divide.
"""

import sys
import numpy as np

if "/opt/trn_rl_repo" not in sys.path:
    sys.path.insert(0, "/opt/trn_rl_repo")

import concourse.bass as bass
import concourse.bacc as bacc
import concourse.tile as tile
from concourse import mybir
from concourse.bass_utils import run_bass_kernel_spmd

F32 = mybir.dt.float32
BF16 = mybir.dt.bfloat16
ALU = mybir.AluOpType

N, L, H, E, M = 2, 8192, 8, 64, 64
C = 128
NCH = L // C            # 64 chunks
NSC = NCH // 2          # 32 superchunks
G = 8                   # chunks per group
SG = G // 2             # superchunks per group
NG = NCH // G           # 8 groups
EPS = 1e-6


def emit(tc, nc, qt2, kt, ks, v2m, o_vc, o_nc):
    with (
        tc.tile_pool(name="const", bufs=1) as const,
        tc.tile_pool(name="big", bufs=1) as big,
    ):
        wub = const.tile([C, C], BF16)          # warm-up weights only
        nc.vector.memset(wub, 0.0)
        iot = const.tile([C, C], mybir.dt.int32)
        nc.gpsimd.iota(iot, pattern=[[1, C]], base=0, channel_multiplier=-1)
        atmask = const.tile([C, 3, C], BF16)    # [tri | ones | tri]
        nc.vector.tensor_scalar(atmask[:, 0, :], iot, 0, None, ALU.is_ge)
        nc.vector.memset(atmask[:, 1, :], 1.0)
        nc.vector.tensor_copy(atmask[:, 2, :], atmask[:, 0, :])

        qt2_t = big.tile([C, L], BF16)
        ktp_t = big.tile([C, 2, L], BF16)
        ks_t = big.tile([C, NCH, C], BF16)
        v2m_t = big.tile([C, NCH, C], BF16)
        SS = big.tile([C, NSC, C], BF16)        # block-diag S snapshots
        SP = big.tile([C, 2, C], F32)           # ping-pong fp32 state
        SfinBD = big.tile([C, C], BF16)
        nc_sb = big.tile([C, L], BF16)
        nc.vector.memset(SP, 0.0)
        nc.gpsimd.memset(SS[:, 0], 0.0)
        # zero halves of ktp (other-head rows must kill the contraction)
        nc.gpsimd.memset(ktp_t[E:C, 0, :], 0.0)
        nc.gpsimd.memset(ktp_t[0:E, 1, :], 0.0)

        def load_kv(a_lo, a_hi):
            sl_a = slice(a_lo, a_hi)
            nc.sync.dma_start(out=ks_t[:, sl_a], in_=ks[:, sl_a])
            nc.sync.dma_start(out=v2m_t[:, sl_a], in_=v2m[:, sl_a])

        def load_q(a_lo, a_hi):
            sl_l = slice(a_lo * C, a_hi * C)
            nc.scalar.dma_start(out=ktp_t[0:E, 0, sl_l], in_=kt[:, 0, sl_l])
            nc.scalar.dma_start(out=ktp_t[E:C, 1, sl_l], in_=kt[:, 1, sl_l])
            nc.scalar.dma_start(out=qt2_t[:, sl_l], in_=qt2[:, sl_l])

        load_kv(0, G)
        load_q(0, G)
        load_kv(G, 2 * G)
        load_q(G, 2 * G)

        with (
            tc.tile_pool(name="atsb", bufs=5) as atsb_pool,
            tc.tile_pool(name="ovb", bufs=3) as ovb_pool,
            tc.tile_pool(name="d_ps", bufs=1, space="PSUM") as d_pool,
            tc.tile_pool(name="at_ps", bufs=2, space="PSUM") as at_pool,
            tc.tile_pool(name="vc_ps", bufs=2, space="PSUM") as vc_pool,
        ):
            # persistent ping-pong D tile: the col-tiled head MMs only
            # clear/write their own diagonal blocks, so the off-blocks
            # are zeroed once here and never touched again
            dd = d_pool.tile([C, 2, C], F32, tag="dd")
            nc.vector.memset(dd, 0.0)
            wudrip = d_pool.tile([C, C], F32, tag="wudrip")

            def drip(n=1):
                for _ in range(n):
                    nc.tensor.matmul(wudrip[0:64, :], lhsT=wub[:, 0:64],
                                     rhs=wub, start=True, stop=True,
                                     skip_group_check=True)

            # HAM warm-up: dense full-mode MMs while the prologue DMA
            # runs; re-warm once the DMA-starved early window has passed
            def warm_burst(n):
                wu = at_pool.tile([C, 2, 512], F32, tag="at")
                for _ in range(n):
                    nc.tensor.matmul(wu[:, 0, 0:128], lhsT=wub, rhs=wub,
                                     start=True, stop=True,
                                     skip_group_check=True)

            warm_burst(64)

            for it in range(NG + 1):
                if 1 <= it <= NG - 2:
                    load_kv((it + 1) * G, (it + 2) * G)
                    load_q((it + 1) * G, (it + 2) * G)
                if it == 2:
                    warm_burst(48)
                g = it - 1
                g0 = g * G

                at_tiles = []

                def at_bank(p2):
                    # superchunk at tile: per head N=256 (diag c0 +
                    # cross c0->c1) then N=128 (diag c1); one DVE evac
                    c0 = g0 + 2 * p2
                    cb0 = slice(c0 * C, (c0 + 1) * C)
                    cb2 = slice(c0 * C, (c0 + 2) * C)
                    cb1 = slice((c0 + 1) * C, (c0 + 2) * C)
                    at_ps = at_pool.tile([C, 2, 512], F32, tag="at")
                    for h in range(2):
                        nc.tensor.matmul(
                            at_ps[:, h, 0:256], lhsT=ktp_t[:, h, cb0],
                            rhs=qt2_t[:, cb2],
                            start=True, stop=False, skip_group_check=True)
                        nc.tensor.matmul(
                            at_ps[:, h, 256:384], lhsT=ktp_t[:, h, cb1],
                            rhs=qt2_t[:, cb1],
                            start=True, stop=(h == 1),
                            skip_group_check=True)
                    at_sb = atsb_pool.tile([C, 2, 3, C], BF16, tag="atsb")
                    atv = at_ps[:, :, 0:384].rearrange(
                        "p h (a c) -> p h a c", a=3)
                    nc.vector.tensor_tensor(
                        at_sb[:, :, 0:3:2, :], atv[:, :, 0:3:2, :],
                        atmask[:, None, 0:3:2, :].broadcast_to(
                            [C, 2, 2, C]),
                        ALU.mult)
                    nc.scalar.copy(out=at_sb[:, :, 1, :],
                                   in_=atv[:, :, 1, :])
                    at_tiles.append(at_sb)

                # ---- D phase (group `it`, per superchunk) with group
                # g's at-banks interleaved to keep the PE stream dense
                if it < NG:
                    for scc in range(SG):
                        c0 = it * G + 2 * scc
                        sc = it * SG + scc
                        sp_p, sp_n = sc % 2, (sc + 1) % 2
                        d = dd[:, sc % 2, :]
                        for j in range(2):
                            c = c0 + j
                            nc.tensor.matmul(
                                d[0:E, 0:E], lhsT=ks_t[:, c, 0:E],
                                rhs=v2m_t[:, c, 0:E], start=(j == 0),
                                stop=(j == 1), skip_group_check=True)
                            nc.tensor.matmul(
                                d[E:C, E:C], lhsT=ks_t[:, c, E:C],
                                rhs=v2m_t[:, c, E:C], start=(j == 0),
                                stop=(j == 1), skip_group_check=True)
                        nc.vector.tensor_tensor(
                            SP[:, sp_n], d, SP[:, sp_p], ALU.add)
                        dst = SfinBD if sc == NSC - 1 else SS[:, sc + 1]
                        if scc % 2 == 0:
                            nc.gpsimd.tensor_copy(dst, SP[:, sp_n])
                        else:
                            nc.scalar.copy(out=dst, in_=SP[:, sp_n])
                        drip()
                        if it >= 1:
                            at_bank(scc)
                else:
                    for p2 in range(SG):
                        at_bank(p2)
                        drip()

                if it == 0:
                    continue

                # ---- inter + intra per superchunk of group g ----
                ovb = ovb_pool.tile([C, 4, C], BF16, tag="ovb")
                ovb2 = ovb_pool.tile([C, 4, C], BF16, tag="ovb")
                for p2 in range(SG):
                    c0 = g0 + 2 * p2
                    sc = g * SG + p2
                    cb2 = slice(c0 * C, (c0 + 2) * C)
                    vc_ps = vc_pool.tile([C, 2, C], F32, tag="vc")
                    vcv = vc_ps.rearrange("p a c -> p (a c)")
                    nc.tensor.matmul(
                        vcv, lhsT=SS[:, sc], rhs=qt2_t[:, cb2],
                        start=True, stop=False, skip_group_check=True)
                    for h in range(2):
                        hb = slice(h * E, (h + 1) * E)
                        nc.tensor.matmul(
                            vcv[hb, :], lhsT=v2m_t[:, c0, hb],
                            rhs=at_tiles[p2][:, h, 0:2, :].rearrange(
                                "p a c -> p (a c)"),
                            start=False, stop=False, skip_group_check=True)
                        nc.tensor.matmul(
                            vcv[hb, 128:256], lhsT=v2m_t[:, c0 + 1, hb],
                            rhs=at_tiles[p2][:, h, 2, :],
                            start=False, stop=(h == 1),
                            skip_group_check=True)
                    dst = ovb if p2 < 2 else ovb2
                    nc.scalar.copy(
                        out=dst[:, 2 * (p2 % 2):2 * (p2 % 2) + 2, :],
                        in_=vc_ps)
                nc.sync.dma_start(out=o_vc[:, g0:g0 + 4], in_=ovb)
                nc.sync.dma_start(out=o_vc[:, g0 + 4:g0 + G], in_=ovb2)

                # ---- nc phase (last iter): SfinBD @ qt2, N=512, two
                # blocks per 2-bank at tile ----
                if it == NG:
                    for bp in range(L // 1024):
                        ncp = at_pool.tile([C, 2, 512], F32, tag="at")
                        ncf = ncp.rearrange("p a c -> p (a c)")
                        for half in range(2):
                            blk = 2 * bp + half
                            lb = slice(blk * 512, (blk + 1) * 512)
                            ncv = ncf[:, half * 512:(half + 1) * 512]
                            nc.tensor.matmul(
                                ncv, lhsT=SfinBD, rhs=qt2_t[:, lb],
                                start=True, stop=True,
                                skip_group_check=True)
                            if half == 0:
                                nc.scalar.copy(out=nc_sb[:, lb], in_=ncv)
                            else:
                                nc.vector.tensor_copy(nc_sb[:, lb], ncv)
                        lq = slice(bp * 1024, (bp + 1) * 1024)
                        nc.sync.dma_start(out=o_nc[:, lq], in_=nc_sb[:, lq])


def build():
    nc = bacc.Bacc("TRN2", target_bir_lowering=False, debug=False)
    qt2 = nc.dram_tensor("qt2", [C, L], BF16, kind="ExternalInput").ap()
    kt = nc.dram_tensor("kt", [E, 2, L], BF16, kind="ExternalInput").ap()
    ks = nc.dram_tensor("ks", [C, NCH, C], BF16, kind="ExternalInput").ap()
    v2m = nc.dram_tensor("v2m", [C, NCH, C], BF16, kind="ExternalInput").ap()
    o_vc = nc.dram_tensor("o_vc", [C, NCH, C], BF16,
                          kind="ExternalOutput").ap()
    o_nc = nc.dram_tensor("o_nc", [C, L], BF16, kind="ExternalOutput").ap()
    with tile.TileContext(nc) as tc:
        emit(tc, nc, qt2, kt, ks, v2m, o_vc, o_nc)
    nc.compile()
    return nc


_NC = None
_last_in_maps = None


def _get_nc():
    global _NC
    if _NC is None:
        _NC = build()
    return _NC


def _bf16(x):
    import ml_dtypes
    return np.ascontiguousarray(x, dtype=np.float32).astype(ml_dtypes.bfloat16)


def _feat(x):
    # elu(x) + 1 in fp32: exp(min(x,0)) + relu(x)
    return np.exp(np.minimum(x, 0.0)) + np.maximum(x, 0.0)


def kernel(queries, keys, values, key_mask):
    global _last_in_maps
    nc = _get_nc()
    queries = np.asarray(queries, dtype=np.float32)
    keys = np.asarray(keys, dtype=np.float32)
    values = np.asarray(values, dtype=np.float32)
    key_mask = np.asarray(key_mask, dtype=np.float32)

    Qf = _feat(queries)
    Kf = _feat(keys) * key_mask[:, :, None, None]
    Vm = values * key_mask[:, :, None, None]
    denc = np.einsum('nlhe,nlhe->nlh', Qf, np.cumsum(Kf, axis=1)) + EPS
    dennc = np.einsum('nlhe,nhe->nlh', Qf, Kf.sum(axis=1)) + EPS

    in_maps = []
    for i in range(8):
        n, h0 = i // 4, 2 * (i % 4)
        qh = Qf[n, :, h0:h0 + 2, :]                   # [L, 2, 64]
        kh = Kf[n, :, h0:h0 + 2, :]
        vh = Vm[n, :, h0:h0 + 2, :]
        qs = qh.transpose(1, 2, 0).reshape(C, L)      # stacked heads
        in_maps.append({
            "qt2": _bf16(qs),
            "kt": _bf16(kh.transpose(2, 1, 0)),
            "ks": _bf16(kh.reshape(NCH, C, C).transpose(1, 0, 2)),
            "v2m": _bf16(vh.reshape(NCH, C, C).transpose(1, 0, 2)),
        })
    _last_in_maps = in_maps
    res = run_bass_kernel_spmd(nc, in_maps, core_ids=list(range(8)))
    V = np.empty((N, L, H, M), np.float32)
    Vc = np.empty((N, L, H, M), np.float32)
    for i in range(8):
        n, h0 = i // 4, 2 * (i % 4)
        ovc = res.results[i]["o_vc"].astype(np.float32)   # [128, NCH, 128]
        onc = res.results[i]["o_nc"].astype(np.float32)   # [128, L]
        num_c = ovc.transpose(1, 2, 0).reshape(L, C)      # [l, (h m)]
        num_n = onc.T                                     # [l, (h m)]
        for h in range(2):
            Vc[n, :, h0 + h, :] = (num_c[:, h * E:(h + 1) * E]
                                   / denc[n, :, h0 + h, None])
            V[n, :, h0 + h, :] = (num_n[:, h * E:(h + 1) * E]
                                  / dennc[n, :, h0 + h, None])
    return (V, Vc)
